# revision 5
# baseline (speedup 1.0000x reference)
"""GPRGNN kernel for Trainium2 (8 NeuronCores, batch-parallel).

Math: the reference collapses algebraically.
  h[b]  = relu(x[b] . w1 + b1)                 (scalar per batch row)
  hid[b,n] = u[n]*h[b] + v[n]                  (rank-1 + bias)
      where u = sum_k temp[k] S^k w2, v = sum_k temp[k] S^k b2,
      S the gcn-normalized adjacency (tiny, 1216 nodes) - computed on host.
  out = log_softmax over 64-wide groups of hid (19 groups of 64).

Device work per core (B_loc=512 rows): read x slice (2.4 MB), compute h via
fused multiply-reduce, build t = u*h+v, grouped log-softmax, write out slice
(2.4 MB).  Memory-bound by design.
"""

import sys

for _p in ("/opt/trn_rl_repo",):
    if _p not in sys.path:
        sys.path.insert(0, _p)

import numpy as np

import concourse.bass as bass
import concourse.bacc as bacc
import concourse.mybir as mybir
from concourse.tile import TileContext
from concourse.bass_utils import run_bass_kernel_spmd

N_NODES = 1216
B = 4096
N_CORES = 8
B_LOC = B // N_CORES  # 512
P = 128
NT = B_LOC // P  # 4 tiles of 128 batch rows per core
G, GS = 19, 64  # softmax groups
F32 = mybir.dt.float32

# test.py hooks
TRACE = False
LAST_RESULT = None

_NC_CACHE = None


def _build_nc():
    nc = bacc.Bacc(None, target_bir_lowering=False)
    x = nc.dram_tensor("x", [B_LOC, N_NODES], F32, kind="ExternalInput")
    # consts rows: 0 = w1, 1 = u, 2 = v
    consts = nc.dram_tensor("consts", [3, N_NODES], F32, kind="ExternalInput")
    b1 = nc.dram_tensor("b1", [1, 1], F32, kind="ExternalInput")
    out = nc.dram_tensor("out", [B_LOC, N_NODES], F32, kind="ExternalOutput")

    with TileContext(nc) as tc:
        with (
            tc.tile_pool(name="singles", bufs=1) as singles,
            tc.tile_pool(name="xp", bufs=3) as xp,
            tc.tile_pool(name="work", bufs=2) as work,
            tc.tile_pool(name="outp", bufs=3) as outp,
            tc.tile_pool(name="small", bufs=4) as small,
        ):
            # Broadcast the three const rows across all 128 partitions.
            cb = singles.tile([P, 3, N_NODES], F32)
            c_ap = consts[:, :]
            src = bass.AP(
                tensor=c_ap.tensor,
                offset=c_ap.offset,
                ap=[[0, P], [N_NODES, 3], [1, N_NODES]],
            )
            nc.sync.dma_start(out=cb, in_=src)
            w1_t = cb[:, 0, :]
            u_t = cb[:, 1, :]
            v_t = cb[:, 2, :]

            b1_t = singles.tile([P, 1], F32)
            b_ap = b1[:, :]
            nc.sync.dma_start(
                out=b1_t,
                in_=bass.AP(tensor=b_ap.tensor, offset=b_ap.offset, ap=[[0, P], [1, 1]]),
            )

            for i in range(NT):
                xt = xp.tile([P, N_NODES], F32)
                nc.sync.dma_start(out=xt, in_=x[i * P : (i + 1) * P, :])

                # h_raw[b] = sum_f x[b,f] * w1[f]
                prod = work.tile([P, N_NODES], F32)
                hraw = small.tile([P, 1], F32)
                nc.vector.tensor_mul(prod, xt, w1_t)
                nc.vector.tensor_reduce(
                    out=hraw, in_=prod, axis=mybir.AxisListType.X,
                    op=mybir.AluOpType.add,
                )
                h = small.tile([P, 1], F32)
                nc.scalar.activation(
                    out=h, in_=hraw, func=mybir.ActivationFunctionType.Relu,
                    bias=b1_t[:, :], scale=1.0,
                )

                # t = u*h + v
                t = work.tile([P, N_NODES], F32)
                nc.scalar.activation(
                    out=t, in_=u_t, func=mybir.ActivationFunctionType.Copy,
                    scale=h[:, :],
                )
                nc.vector.tensor_add(t, t, v_t)

                # e = exp(t); s = group sums; L = log(s)
                e = work.tile([P, N_NODES], F32)
                nc.scalar.activation(
                    out=e, in_=t, func=mybir.ActivationFunctionType.Exp
                )
                s = small.tile([P, G], F32)
                nc.vector.tensor_reduce(
                    out=s,
                    in_=e[:, :].rearrange("p (g s) -> p g s", s=GS),
                    axis=mybir.AxisListType.X,
                    op=mybir.AluOpType.add,
                )
                L = small.tile([P, G], F32)
                nc.scalar.activation(
                    out=L, in_=s, func=mybir.ActivationFunctionType.Ln
                )

                # out = t - L (broadcast over the 64 in-group positions)
                o = outp.tile([P, N_NODES], F32)
                L_ap = L[:, :]
                Lb = bass.AP(
                    tensor=L_ap.tensor,
                    offset=L_ap.offset,
                    ap=[L_ap.ap[0], L_ap.ap[1], [0, GS]],
                )
                nc.vector.tensor_sub(
                    o[:, :].rearrange("p (g s) -> p g s", s=GS),
                    t[:, :].rearrange("p (g s) -> p g s", s=GS),
                    Lb,
                )
                nc.sync.dma_start(out=out[i * P : (i + 1) * P, :], in_=o)
    return nc


def _graph_vectors(edge_index, lin2_w, lin2_b, temp):
    """u = sum_k temp[k] S^k w2 ; v = sum_k temp[k] S^k b2 (host, tiny)."""
    N = N_NODES
    ei = np.asarray(edge_index)
    loop = np.arange(N, dtype=ei.dtype)
    row = np.concatenate([ei[0], loop])
    col = np.concatenate([ei[1], loop])
    deg = np.bincount(col, minlength=N).astype(np.float64)
    dinv = np.where(deg > 0, 1.0 / np.sqrt(np.maximum(deg, 1e-12)), 0.0)
    norm = dinv[row] * dinv[col]

    temp = np.asarray(temp, dtype=np.float64)
    w2 = np.asarray(lin2_w, dtype=np.float64)[:, 0]
    b2 = np.asarray(lin2_b, dtype=np.float64)
    u = temp[0] * w2
    v = temp[0] * b2
    yu, yv = w2, b2
    for k in range(len(temp) - 1):
        nyu = np.zeros(N)
        np.add.at(nyu, col, norm * yu[row])
        nyv = np.zeros(N)
        np.add.at(nyv, col, norm * yv[row])
        yu, yv = nyu, nyv
        u += temp[k + 1] * yu
        v += temp[k + 1] * yv
    return u.astype(np.float32), v.astype(np.float32)


def kernel(x, edge_index, lin1_w, lin1_b, lin2_w, lin2_b, temp):
    global _NC_CACHE, LAST_RESULT
    x = np.ascontiguousarray(np.asarray(x, dtype=np.float32))
    u, v = _graph_vectors(edge_index, lin2_w, lin2_b, temp)
    w1 = np.asarray(lin1_w, dtype=np.float32)[0]
    consts = np.ascontiguousarray(np.stack([w1, u, v]).astype(np.float32))
    b1 = np.array([[np.asarray(lin1_b, dtype=np.float32).reshape(-1)[0]]],
                  dtype=np.float32)

    if _NC_CACHE is None:
        nc = _build_nc()
        nc.finalize()
        _NC_CACHE = nc
    nc = _NC_CACHE

    in_maps = [
        {"x": x[c * B_LOC : (c + 1) * B_LOC], "consts": consts, "b1": b1}
        for c in range(N_CORES)
    ]
    res = run_bass_kernel_spmd(
        nc, in_maps, core_ids=list(range(N_CORES)), trace=TRACE
    )
    LAST_RESULT = res
    out = np.concatenate([r["out"] for r in res.results], axis=0)
    return out.reshape(B, 1, G, GS)
